# revision 16
# baseline (speedup 1.0000x reference)
"""Trainium2 Bass kernel for nn_AdaptiveCapsule (capsule routing).

Strategy (8 NeuronCores, shard in_caps I=1024 -> IL=128 per core):
  u_hat[b,i,o,d] = sum_e W[i,o,d,e] * x[b,i,e]   (34 GFLOP einsum)
  3 routing iterations over out_caps with tiny (64x512 f32) AllReduces.

v7 design:
  - Host packs W and x to fp16 (10 mantissa bits: ~4x less quantization
    noise than bf16) in the exact transposed SBUF layout (contraction
    axis E on partitions), so the device uses plain large HWDGE DMAs
    alternating both rings (sync/scalar): 32x 2MiB for W + 32x 256KiB
    for x. No X-bar transpose, half the bytes of the u16 pipeline.
  - TensorE: per in-cap pair, col-tiled (M=64) fp16 matmuls at
    tile_position (0,0)/(0,64), K=128 fully live, 4 K-chunks into one
    (128,512) f32 PSUM tile; the two column halves stream concurrently.
    Fold matmul (ones-stack lhsT) accumulates sum_i u_hat for routing
    iteration 0 (uniform attn), delayed one pair so the PE never stalls
    on the PSUM->SBUF drain (all drains on VectorE so the scalar ring
    never queues a drain in front of a W DMA).
  - u_hat kept in SBUF fp16 as (128, pair, D*O) with free axis in
    d-major order (od = d*O + o). That makes BOTH routing multiplies
    DVE 2x-mode eligible: v broadcast is middle-axis (inner step 1) and
    attn broadcast is over d (o contiguous inner). The d-reduction for
    logits runs as in-place fp16 tree-adds over the middle axis (2x),
    final stage emitting f32 logits.
  - softmax: per-(b,i) max-sub (tail logits overflow exp otherwise),
    exp on ScalarE, attn normalized to fp16.
  - s = sum_i attn*u_hat via A/B col-tiled fold matmuls (even pairs ->
    psum[0:64], odd -> psum[64:128]) plus one final f32 fold, then
    AllReduce (64x512 f32) via internal DRAM tiles; the result loads
    into both partition halves and squash runs on all 128 partitions.
  - Output leaves in d-major order; the host transposes back to (B,O,D).
"""

import sys

sys.path.insert(0, "/opt/trn_rl_repo")

import numpy as np

from concourse import bacc, bass, tile
from concourse import mybir
from concourse.bass_utils import run_bass_kernel_spmd

NCORES = 8
B, I, O, D, E = 64, 1024, 16, 32, 512
IL = I // NCORES  # 128 in_caps per core
OD = O * D  # 512
PAIRS = IL // 2  # 64 i-pairs per core
KCH = E // 128  # 4 contraction chunks of 128
GP = 4  # pairs per W DMA group (4 MiB per DMA)
NG = PAIRS // GP  # 32 groups
BLK = 16  # pairs per routing DVE block
NBLK = PAIRS // BLK
F32 = mybir.dt.float32
FP16 = mybir.dt.float16
AX = mybir.AxisListType
ALU = mybir.AluOpType
ACTF = mybir.ActivationFunctionType

_CACHE = {}


def _build():
    nc = bacc.Bacc(None, target_bir_lowering=False, num_devices=NCORES)
    wp = nc.dram_tensor("wp", [NG, 128, GP * 2 * KCH * OD], FP16, kind="ExternalInput")
    xp = nc.dram_tensor("xp", [NG, 128, GP * 2 * KCH * B], FP16, kind="ExternalInput")
    fin = nc.dram_tensor("fold", [128, 64], FP16, kind="ExternalInput")
    out = nc.dram_tensor("out", [B, OD], F32, kind="ExternalOutput")
    rg = [list(range(NCORES))]

    with tile.TileContext(nc) as tc:
        with (
            tc.tile_pool(name="persist", bufs=1) as persist,
            tc.tile_pool(name="wt", bufs=2) as wpool,
            tc.tile_pool(name="xt", bufs=3) as xpool,
            tc.tile_pool(name="tmp", bufs=2) as tmpool,
            tc.tile_pool(name="psum", bufs=3, space="PSUM") as psum,
            tc.tile_pool(name="psacc", bufs=2, space="PSUM") as psacc,
            tc.tile_pool(name="psfold", bufs=2, space="PSUM") as psfold,
            tc.tile_pool(name="dram", bufs=6, space="DRAM") as dram,
        ):
            uhat = persist.tile([128, PAIRS, OD], FP16)
            foldb = persist.tile([128, 64], FP16)
            foldf = persist.tile([128, 64], F32)
            nc.sync.dma_start(foldb[:], fin[:])
            nc.vector.tensor_copy(foldf[:], foldb[:])
            acc_ps = psacc.tile([64, OD], F32, tag="accps")

            # ---- Phase 1: u_hat via pre-transposed fp16 matmuls ----
            prev = None
            for g in range(NG):
                ring_w = nc.sync if g % 2 == 0 else nc.scalar
                ring_x = nc.scalar if g % 2 == 0 else nc.sync
                wt = wpool.tile([128, GP * 2, KCH, OD], FP16, tag="wt")
                ring_w.dma_start(wt[:], wp[g])
                xt = xpool.tile([128, GP * 2, KCH, B], FP16, tag="xt")
                ring_x.dma_start(xt[:], xp[g])
                for lp in range(GP):
                    p = g * GP + lp
                    ps = psum.tile([128, OD], F32, tag="ps")
                    for c in range(KCH):
                        nc.tensor.matmul(
                            ps[0:64, :], xt[:, 2 * lp, c, :], wt[:, 2 * lp, c, :],
                            start=(c == 0), stop=(c == KCH - 1),
                            tile_position=(0, 0),
                        )
                        nc.tensor.matmul(
                            ps[64:128, :], xt[:, 2 * lp + 1, c, :],
                            wt[:, 2 * lp + 1, c, :],
                            start=(c == 0), stop=(c == KCH - 1),
                            tile_position=(0, 64),
                        )
                    nc.vector.tensor_copy(uhat[:, p, :], ps[:])
                    # fold (i-sum for iter-0 s) delayed one pair so the PE
                    # never waits on the drain of the pair it just computed
                    if prev is not None:
                        nc.tensor.matmul(
                            acc_ps[:], foldb[:], uhat[:, prev, :],
                            start=(prev == 0), stop=False,
                        )
                    prev = p
            nc.tensor.matmul(
                acc_ps[:], foldb[:], uhat[:, prev, :], start=False, stop=True
            )

            # ---- Routing ----
            logits = persist.tile([128, PAIRS, O], F32)
            red = persist.tile([128, PAIRS, O], F32)
            attn32 = persist.tile([128, PAIRS, O], F32)
            attn_h = persist.tile([128, PAIRS, O], FP16)
            sm = persist.tile([128, PAIRS], F32)
            mx = persist.tile([128, PAIRS], F32)
            accsb = persist.tile([128, OD], F32)
            sprep = persist.tile([128, OD], F32)
            v_rep = persist.tile([128, OD], FP16)
            s_sb = persist.tile([64, OD], F32)
            sq = persist.tile([128, OD], F32)
            nrm = persist.tile([128, O], F32)
            den = persist.tile([128, O], F32)

            def logits_block(blk, it):
                # tmp = u_hat * v (2x), then tree-add over d (2x, in-place),
                # final stage writes f32 logits
                sl = slice(blk * BLK, (blk + 1) * BLK)
                tmp = tmpool.tile([128, BLK, OD], FP16, tag="tmp", name=f"tl{blk}")
                nc.vector.tensor_tensor(
                    tmp[:], uhat[:, sl, :],
                    v_rep[:].unsqueeze(1).broadcast_to((128, BLK, OD)),
                    op=ALU.mult,
                )
                v4 = tmp[:].rearrange("p a (d o) -> p a d o", d=D)
                w = D
                while w > 2:
                    h = w // 2
                    nc.vector.tensor_tensor(
                        v4[:, :, 0:h, :], v4[:, :, 0:h, :], v4[:, :, h:w, :],
                        op=ALU.add,
                    )
                    w = h
                tgt = logits if it == 1 else red
                nc.vector.tensor_tensor(
                    tgt[:, sl, :], v4[:, :, 0, :], v4[:, :, 1, :], op=ALU.add
                )

            def s_block(blk, acc2):
                # tmp = u_hat * attn (2x: o is the contiguous inner axis),
                # then A/B col-tiled fold matmuls accumulate sum_i
                sl = slice(blk * BLK, (blk + 1) * BLK)
                tmp = tmpool.tile([128, BLK, OD], FP16, tag="tmp", name=f"ts{blk}")
                nc.vector.tensor_tensor(
                    tmp[:].rearrange("p a (d o) -> p a d o", d=D),
                    uhat[:, sl, :].rearrange("p a (d o) -> p a d o", d=D),
                    attn_h[:, sl, :].unsqueeze(2).broadcast_to((128, BLK, D, O)),
                    op=ALU.mult,
                )
                for lp in range(BLK):
                    p = blk * BLK + lp
                    half = acc2[0:64, :] if p % 2 == 0 else acc2[64:128, :]
                    nc.tensor.matmul(
                        half, foldb[:], tmp[:, lp, :],
                        start=(p < 2), stop=(p >= PAIRS - 2),
                        tile_position=((0, 0) if p % 2 == 0 else (0, 64)),
                    )

            for it in range(3):
                if it > 0:
                    for blk in range(NBLK):
                        logits_block(blk, it)
                    if it == 2:
                        nc.vector.tensor_tensor(
                            logits[:], logits[:], red[:], op=ALU.add
                        )
                    # softmax over o: per-(b,i) max-sub is required --
                    # measured logits reach +290/-245 by iteration 2, so no
                    # constant shift keeps exp in f32 range for all items
                    nc.vector.tensor_reduce(
                        mx[:], logits[:], axis=AX.X, op=ALU.max
                    )
                    nc.vector.tensor_tensor(
                        red[:], logits[:],
                        mx[:].unsqueeze(-1).broadcast_to((128, PAIRS, O)),
                        op=ALU.subtract,
                    )
                    nc.scalar.activation(attn32[:], red[:], ACTF.Exp)
                    nc.vector.tensor_reduce(
                        sm[:], attn32[:], axis=AX.X, op=ALU.add
                    )
                    nc.vector.reciprocal(sm[:], sm[:])
                    nc.vector.tensor_tensor(
                        attn_h[:], attn32[:],
                        sm[:].unsqueeze(-1).broadcast_to((128, PAIRS, O)),
                        op=ALU.mult,
                    )
                    acc2 = psacc.tile([128, OD], F32, tag="accps")
                    for blk in range(NBLK):
                        s_block(blk, acc2)
                    nc.vector.tensor_copy(accsb[:], acc2[:])
                    s_ps = psfold.tile([64, OD], F32, tag="fold")
                    nc.tensor.matmul(
                        s_ps[:], foldf[:], accsb[:], start=True, stop=True
                    )
                    nc.scalar.mul(s_sb[:], s_ps[:], 1.0)
                else:
                    nc.scalar.mul(s_sb[:], acc_ps[:], 1.0 / O)

                cin = dram.tile([64, OD], F32, tag="cin")
                cout = dram.tile([64, OD], F32, tag="cout")
                nc.sync.dma_start(cin[:], s_sb[:])
                nc.gpsimd.collective_compute(
                    "AllReduce", ALU.add, replica_groups=rg,
                    ins=[cin[:].opt()], outs=[cout[:].opt()],
                )
                if it == 2:
                    # final iteration: ship raw s_2; host applies squash
                    nc.sync.dma_start(s_sb[:], cout[:])
                    nc.sync.dma_start(out[:], s_sb[:])
                    break
                nc.sync.dma_start(sprep[0:64, :], cout[:])
                nc.scalar.dma_start(sprep[64:128, :], cout[:])

                # squash(s) = norm/(1+norm^2) * s along d (d-major layout:
                # square then tree-add over the middle d axis)
                nc.vector.tensor_tensor(sq[:], sprep[:], sprep[:], op=ALU.mult)
                sqv = sq[:].rearrange("p (d o) -> p d o", d=D)
                w = D
                while w > 1:
                    h = w // 2
                    nc.vector.tensor_tensor(
                        sqv[:, 0:h, :], sqv[:, 0:h, :], sqv[:, h:w, :],
                        op=ALU.add,
                    )
                    w = h
                n2 = sqv[:, 0, :]  # (128, O)
                nc.scalar.activation(nrm[:], n2, ACTF.Sqrt)
                nc.vector.tensor_scalar_add(den[:], n2, 1.0)
                nc.vector.reciprocal(den[:], den[:])
                nc.vector.tensor_tensor(nrm[:], nrm[:], den[:], op=ALU.mult)
                nc.vector.tensor_tensor(
                    v_rep[:].rearrange("p (d o) -> p d o", d=D),
                    sprep[:].rearrange("p (d o) -> p d o", d=D),
                    nrm[:].unsqueeze(1).broadcast_to((128, D, O)),
                    op=ALU.mult,
                )

    nc.compile()
    return nc


def _get_nc():
    if "nc" not in _CACHE:
        _CACHE["nc"] = _build()
    return _CACHE["nc"]


def _pack_w(w_shard):
    # (IL, OD_do, E) f32 (already d-major) -> fp16 (NG, 128, GP*2*KCH*OD)
    wb = w_shard.astype(np.float16)
    wb = wb.reshape(IL, OD, KCH, 128)
    wb = wb.transpose(0, 3, 2, 1)  # (i, part, c, od)
    wb = wb.reshape(NG, GP * 2, 128, KCH, OD)
    wb = np.ascontiguousarray(wb.transpose(0, 2, 1, 3, 4))
    return wb.reshape(NG, 128, GP * 2 * KCH * OD)


def _pack_x(x_shard):
    # (B, IL, E) f32 -> fp16 (NG, 128, GP*2*KCH*B), partition = e%128
    xb = x_shard.astype(np.float16)
    xb = xb.reshape(B, IL, KCH, 128)
    xb = xb.transpose(1, 3, 2, 0)  # (i, part, c, b)
    xb = xb.reshape(NG, GP * 2, 128, KCH, B)
    xb = np.ascontiguousarray(xb.transpose(0, 2, 1, 3, 4))
    return xb.reshape(NG, 128, GP * 2 * KCH * B)


def _prep_inputs(x, W, route_bias):
    x = np.ascontiguousarray(np.asarray(x, dtype=np.float32))
    W = np.asarray(W, dtype=np.float32)
    rb = np.asarray(route_bias, dtype=np.float32)
    if np.any(rb):
        W = W + rb  # reference adds the (1,1,O,1,1) bias onto W
    # d-major free axis: od = d*O + o
    W0 = np.ascontiguousarray(
        W.reshape(I, O, D, E).transpose(0, 2, 1, 3).reshape(I, OD, E)
    )
    foldm = np.vstack([np.eye(64), np.eye(64)]).astype(np.float16)
    in_maps = []
    for r in range(NCORES):
        sl = slice(r * IL, (r + 1) * IL)
        in_maps.append(
            {
                "wp": _pack_w(np.ascontiguousarray(W0[sl])),
                "xp": _pack_x(np.ascontiguousarray(x[:, sl, :])),
                "fold": foldm,
            }
        )
    return in_maps


def kernel(x, W, route_bias, _trace=False, _trace_kwargs=None):
    in_maps = _prep_inputs(x, W, route_bias)
    res = run_bass_kernel_spmd(
        _get_nc(), in_maps, core_ids=list(range(NCORES)),
        trace=_trace, **(_trace_kwargs or {}),
    )
    _CACHE["last_results"] = res
    # device output is raw s_2 in d-major order: (B, D, O) -> (B, O, D),
    # then apply the final squash here (tiny: 64x16x32)
    s2 = np.asarray(res.results[0]["out"], dtype=np.float32).reshape(B, D, O)
    s2 = np.ascontiguousarray(s2.transpose(0, 2, 1))
    n = np.linalg.norm(s2, axis=-1, keepdims=True)
    return (n / (1.0 + n * n) * s2).astype(np.float32)


# revision 17
# speedup vs baseline: 1.0633x; 1.0633x over previous
"""Trainium2 Bass kernel for nn_AdaptiveCapsule (capsule routing).

Strategy (8 NeuronCores, shard in_caps I=1024 -> IL=128 per core):
  u_hat[b,i,o,d] = sum_e W[i,o,d,e] * x[b,i,e]   (34 GFLOP einsum)
  3 routing iterations over out_caps with tiny (64x512 f32) AllReduces.

v7 design:
  - Host packs W and x to fp16 (10 mantissa bits: ~4x less quantization
    noise than bf16) in the exact transposed SBUF layout (contraction
    axis E on partitions), so the device uses plain large HWDGE DMAs
    alternating both rings (sync/scalar): 32x 2MiB for W + 32x 256KiB
    for x. No X-bar transpose, half the bytes of the u16 pipeline.
  - TensorE: per in-cap pair, col-tiled (M=64) fp16 matmuls at
    tile_position (0,0)/(0,64), K=128 fully live, 4 K-chunks into one
    (128,512) f32 PSUM tile; the two column halves stream concurrently.
    Fold matmul (ones-stack lhsT) accumulates sum_i u_hat for routing
    iteration 0 (uniform attn), delayed one pair so the PE never stalls
    on the PSUM->SBUF drain (all drains on VectorE so the scalar ring
    never queues a drain in front of a W DMA).
  - u_hat kept in SBUF fp16 as (128, pair, D*O) with free axis in
    d-major order (od = d*O + o). That makes BOTH routing multiplies
    DVE 2x-mode eligible: v broadcast is middle-axis (inner step 1) and
    attn broadcast is over d (o contiguous inner). The d-reduction for
    logits runs as in-place fp16 tree-adds over the middle axis (2x),
    final stage emitting f32 logits.
  - softmax: per-(b,i) max-sub (tail logits overflow exp otherwise),
    exp on ScalarE, attn normalized to fp16.
  - s = sum_i attn*u_hat via A/B col-tiled fold matmuls (even pairs ->
    psum[0:64], odd -> psum[64:128]) plus one final f32 fold, then
    AllReduce (64x512 f32) via internal DRAM tiles; the result loads
    into both partition halves and squash runs on all 128 partitions.
  - Output leaves in d-major order; the host transposes back to (B,O,D).
"""

import sys

sys.path.insert(0, "/opt/trn_rl_repo")

import numpy as np

from concourse import bacc, bass, tile
from concourse import mybir
from concourse.bass_utils import run_bass_kernel_spmd

NCORES = 8
B, I, O, D, E = 64, 1024, 16, 32, 512
IL = I // NCORES  # 128 in_caps per core
OD = O * D  # 512
PAIRS = IL // 2  # 64 i-pairs per core
KCH = E // 128  # 4 contraction chunks of 128
GP = 4  # pairs per W DMA group (4 MiB per DMA)
NG = PAIRS // GP  # 32 groups
BLK = 16  # pairs per routing DVE block
NBLK = PAIRS // BLK
F32 = mybir.dt.float32
FP16 = mybir.dt.float16
AX = mybir.AxisListType
ALU = mybir.AluOpType
ACTF = mybir.ActivationFunctionType

_CACHE = {}


def _build():
    nc = bacc.Bacc(None, target_bir_lowering=False, num_devices=NCORES)
    wp = nc.dram_tensor("wp", [NG, 128, GP * 2 * KCH * OD], FP16, kind="ExternalInput")
    xp = nc.dram_tensor(
        "xp", [NG // 2, 128, 2 * GP * 2 * KCH * B], FP16, kind="ExternalInput"
    )
    fin = nc.dram_tensor("fold", [128, 64], FP16, kind="ExternalInput")
    out = nc.dram_tensor("out", [B, OD], F32, kind="ExternalOutput")
    rg = [list(range(NCORES))]

    with tile.TileContext(nc) as tc:
        with (
            tc.tile_pool(name="persist", bufs=1) as persist,
            tc.tile_pool(name="wt", bufs=2) as wpool,
            tc.tile_pool(name="xt", bufs=2) as xpool,
            tc.tile_pool(name="tmp", bufs=2) as tmpool,
            tc.tile_pool(name="psum", bufs=3, space="PSUM") as psum,
            tc.tile_pool(name="psacc", bufs=2, space="PSUM") as psacc,
            tc.tile_pool(name="psfold", bufs=2, space="PSUM") as psfold,
            tc.tile_pool(name="dram", bufs=6, space="DRAM") as dram,
        ):
            uhat = persist.tile([128, PAIRS, OD], FP16)
            foldb = persist.tile([128, 64], FP16)
            foldf = persist.tile([128, 64], F32)
            nc.sync.dma_start(foldb[:], fin[:])
            nc.vector.tensor_copy(foldf[:], foldb[:])
            acc_ps = psacc.tile([64, OD], F32, tag="accps")

            # warm up the collective path during phase 1: the first real
            # AllReduce otherwise pays a ~20us cold-start on top of skew
            dmy_i = dram.tile([64, 16], FP16, tag="dmy_i")
            dmy_o = dram.tile([64, 16], FP16, tag="dmy_o")
            nc.sync.dma_start(dmy_i[:], foldb[0:64, 0:16])
            nc.gpsimd.collective_compute(
                "AllReduce", ALU.add, replica_groups=rg,
                ins=[dmy_i[:].opt()], outs=[dmy_o[:].opt()],
            )

            # ---- Phase 1: u_hat via pre-transposed fp16 matmuls ----
            prev = None
            for g in range(NG):
                ring_w = nc.sync if g % 2 == 0 else nc.scalar
                ring_x = nc.scalar if g % 2 == 0 else nc.sync
                wt = wpool.tile([128, GP * 2, KCH, OD], FP16, tag="wt")
                ring_w.dma_start(wt[:], wp[g])
                if g % 2 == 0:
                    xt = xpool.tile([128, 2, GP * 2, KCH, B], FP16, tag="xt")
                    ring_x.dma_start(xt[:], xp[g // 2])
                xg = xt[:, g % 2]
                for lp in range(GP):
                    p = g * GP + lp
                    ps = psum.tile([128, OD], F32, tag="ps")
                    for c in range(KCH):
                        nc.tensor.matmul(
                            ps[0:64, :], xg[:, 2 * lp, c, :], wt[:, 2 * lp, c, :],
                            start=(c == 0), stop=(c == KCH - 1),
                            tile_position=(0, 0),
                        )
                        nc.tensor.matmul(
                            ps[64:128, :], xg[:, 2 * lp + 1, c, :],
                            wt[:, 2 * lp + 1, c, :],
                            start=(c == 0), stop=(c == KCH - 1),
                            tile_position=(0, 64),
                        )
                    nc.vector.tensor_copy(uhat[:, p, :], ps[:])
                    # fold (i-sum for iter-0 s) delayed one pair so the PE
                    # never waits on the drain of the pair it just computed
                    if prev is not None:
                        nc.tensor.matmul(
                            acc_ps[:], foldb[:], uhat[:, prev, :],
                            start=(prev == 0), stop=False,
                        )
                    prev = p
            nc.tensor.matmul(
                acc_ps[:], foldb[:], uhat[:, prev, :], start=False, stop=True
            )

            # ---- Routing ----
            logits = persist.tile([128, PAIRS, O], F32)
            red = persist.tile([128, PAIRS, O], F32)
            attn32 = persist.tile([128, PAIRS, O], F32)
            attn_h = persist.tile([128, PAIRS, O], FP16)
            sm = persist.tile([128, PAIRS], F32)
            mx = persist.tile([128, PAIRS], F32)
            accsb = persist.tile([128, OD], F32)
            sprep = persist.tile([128, OD], F32)
            v_rep = persist.tile([128, OD], FP16)
            s_sb = persist.tile([64, OD], F32)
            sq = persist.tile([128, OD], F32)
            nrm = persist.tile([128, O], F32)
            den = persist.tile([128, O], F32)

            def logits_block(blk, it):
                # tmp = u_hat * v (2x), then tree-add over d (2x, in-place),
                # final stage writes f32 logits
                sl = slice(blk * BLK, (blk + 1) * BLK)
                tmp = tmpool.tile([128, BLK, OD], FP16, tag="tmp", name=f"tl{blk}")
                nc.vector.tensor_tensor(
                    tmp[:], uhat[:, sl, :],
                    v_rep[:].unsqueeze(1).broadcast_to((128, BLK, OD)),
                    op=ALU.mult,
                )
                v4 = tmp[:].rearrange("p a (d o) -> p a d o", d=D)
                w = D
                while w > 2:
                    h = w // 2
                    nc.vector.tensor_tensor(
                        v4[:, :, 0:h, :], v4[:, :, 0:h, :], v4[:, :, h:w, :],
                        op=ALU.add,
                    )
                    w = h
                tgt = logits if it == 1 else red
                nc.vector.tensor_tensor(
                    tgt[:, sl, :], v4[:, :, 0, :], v4[:, :, 1, :], op=ALU.add
                )

            def s_block(blk, acc2):
                # tmp = u_hat * attn (2x: o is the contiguous inner axis),
                # then A/B col-tiled fold matmuls accumulate sum_i
                sl = slice(blk * BLK, (blk + 1) * BLK)
                tmp = tmpool.tile([128, BLK, OD], FP16, tag="tmp", name=f"ts{blk}")
                nc.vector.tensor_tensor(
                    tmp[:].rearrange("p a (d o) -> p a d o", d=D),
                    uhat[:, sl, :].rearrange("p a (d o) -> p a d o", d=D),
                    attn_h[:, sl, :].unsqueeze(2).broadcast_to((128, BLK, D, O)),
                    op=ALU.mult,
                )
                for lp in range(BLK):
                    p = blk * BLK + lp
                    half = acc2[0:64, :] if p % 2 == 0 else acc2[64:128, :]
                    nc.tensor.matmul(
                        half, foldb[:], tmp[:, lp, :],
                        start=(p < 2), stop=(p >= PAIRS - 2),
                        tile_position=((0, 0) if p % 2 == 0 else (0, 64)),
                    )

            for it in range(3):
                if it > 0:
                    for blk in range(NBLK):
                        logits_block(blk, it)
                    if it == 2:
                        nc.vector.tensor_tensor(
                            logits[:], logits[:], red[:], op=ALU.add
                        )
                    # softmax over o: per-(b,i) max-sub is required --
                    # measured logits reach +290/-245 by iteration 2, so no
                    # constant shift keeps exp in f32 range for all items
                    nc.vector.tensor_reduce(
                        mx[:], logits[:], axis=AX.X, op=ALU.max
                    )
                    nc.vector.tensor_tensor(
                        red[:], logits[:],
                        mx[:].unsqueeze(-1).broadcast_to((128, PAIRS, O)),
                        op=ALU.subtract,
                    )
                    nc.scalar.activation(attn32[:], red[:], ACTF.Exp)
                    nc.vector.tensor_reduce(
                        sm[:], attn32[:], axis=AX.X, op=ALU.add
                    )
                    nc.vector.reciprocal(sm[:], sm[:])
                    nc.vector.tensor_tensor(
                        attn_h[:], attn32[:],
                        sm[:].unsqueeze(-1).broadcast_to((128, PAIRS, O)),
                        op=ALU.mult,
                    )
                    acc2 = psacc.tile([128, OD], F32, tag="accps")
                    for blk in range(NBLK):
                        s_block(blk, acc2)
                    nc.vector.tensor_copy(accsb[:], acc2[:])
                    s_ps = psfold.tile([64, OD], F32, tag="fold")
                    nc.tensor.matmul(
                        s_ps[:], foldf[:], accsb[:], start=True, stop=True
                    )
                    nc.scalar.mul(s_sb[:], s_ps[:], 1.0)
                else:
                    nc.scalar.mul(s_sb[:], acc_ps[:], 1.0 / O)

                cin = dram.tile([64, OD], F32, tag="cin")
                cout = dram.tile([64, OD], F32, tag="cout")
                nc.sync.dma_start(cin[:], s_sb[:])
                nc.gpsimd.collective_compute(
                    "AllReduce", ALU.add, replica_groups=rg,
                    ins=[cin[:].opt()], outs=[cout[:].opt()],
                )
                if it == 2:
                    # final iteration: ship raw s_2; host applies squash
                    nc.sync.dma_start(out[:], cout[:])
                    break
                nc.sync.dma_start(sprep[0:64, :], cout[:])
                nc.scalar.dma_start(sprep[64:128, :], cout[:])

                # squash(s) = norm/(1+norm^2) * s along d (d-major layout:
                # square then tree-add over the middle d axis)
                nc.vector.tensor_tensor(sq[:], sprep[:], sprep[:], op=ALU.mult)
                sqv = sq[:].rearrange("p (d o) -> p d o", d=D)
                w = D
                while w > 1:
                    h = w // 2
                    nc.vector.tensor_tensor(
                        sqv[:, 0:h, :], sqv[:, 0:h, :], sqv[:, h:w, :],
                        op=ALU.add,
                    )
                    w = h
                n2 = sqv[:, 0, :]  # (128, O)
                nc.scalar.activation(nrm[:], n2, ACTF.Sqrt)
                nc.vector.tensor_scalar_add(den[:], n2, 1.0)
                nc.vector.reciprocal(den[:], den[:])
                nc.vector.tensor_tensor(nrm[:], nrm[:], den[:], op=ALU.mult)
                nc.vector.tensor_tensor(
                    v_rep[:].rearrange("p (d o) -> p d o", d=D),
                    sprep[:].rearrange("p (d o) -> p d o", d=D),
                    nrm[:].unsqueeze(1).broadcast_to((128, D, O)),
                    op=ALU.mult,
                )

    nc.compile()
    return nc


def _get_nc():
    if "nc" not in _CACHE:
        _CACHE["nc"] = _build()
    return _CACHE["nc"]


def _pack_w(w_shard):
    # (IL, OD_do, E) f32 (already d-major) -> fp16 (NG, 128, GP*2*KCH*OD)
    wb = w_shard.astype(np.float16)
    wb = wb.reshape(IL, OD, KCH, 128)
    wb = wb.transpose(0, 3, 2, 1)  # (i, part, c, od)
    wb = wb.reshape(NG, GP * 2, 128, KCH, OD)
    wb = np.ascontiguousarray(wb.transpose(0, 2, 1, 3, 4))
    return wb.reshape(NG, 128, GP * 2 * KCH * OD)


def _pack_x(x_shard):
    # (B, IL, E) f32 -> fp16 (NG, 128, GP*2*KCH*B), partition = e%128
    xb = x_shard.astype(np.float16)
    xb = xb.reshape(B, IL, KCH, 128)
    xb = xb.transpose(1, 3, 2, 0)  # (i, part, c, b)
    xb = xb.reshape(NG // 2, 2 * GP * 2, 128, KCH, B)
    xb = np.ascontiguousarray(xb.transpose(0, 2, 1, 3, 4))
    return xb.reshape(NG // 2, 128, 2 * GP * 2 * KCH * B)


def _prep_inputs(x, W, route_bias):
    x = np.ascontiguousarray(np.asarray(x, dtype=np.float32))
    W = np.asarray(W, dtype=np.float32)
    rb = np.asarray(route_bias, dtype=np.float32)
    if np.any(rb):
        W = W + rb  # reference adds the (1,1,O,1,1) bias onto W
    # d-major free axis: od = d*O + o
    W0 = np.ascontiguousarray(
        W.reshape(I, O, D, E).transpose(0, 2, 1, 3).reshape(I, OD, E)
    )
    foldm = np.vstack([np.eye(64), np.eye(64)]).astype(np.float16)
    in_maps = []
    for r in range(NCORES):
        sl = slice(r * IL, (r + 1) * IL)
        in_maps.append(
            {
                "wp": _pack_w(np.ascontiguousarray(W0[sl])),
                "xp": _pack_x(np.ascontiguousarray(x[:, sl, :])),
                "fold": foldm,
            }
        )
    return in_maps


def kernel(x, W, route_bias, _trace=False, _trace_kwargs=None):
    in_maps = _prep_inputs(x, W, route_bias)
    res = run_bass_kernel_spmd(
        _get_nc(), in_maps, core_ids=list(range(NCORES)),
        trace=_trace, **(_trace_kwargs or {}),
    )
    _CACHE["last_results"] = res
    # device output is raw s_2 in d-major order: (B, D, O) -> (B, O, D),
    # then apply the final squash here (tiny: 64x16x32)
    s2 = np.asarray(res.results[0]["out"], dtype=np.float32).reshape(B, D, O)
    s2 = np.ascontiguousarray(s2.transpose(0, 2, 1))
    n = np.linalg.norm(s2, axis=-1, keepdims=True)
    return (n / (1.0 + n * n) * s2).astype(np.float32)
